# revision 21
# baseline (speedup 1.0000x reference)
"""Trainium2 Bass kernel for causal MultiHeadAttention (B=4,S=2048,E=1024,H=16).

Sharding: 8 cores = (batch b, head-half) grid. Core c handles batch c//2 and
heads [8*(c%2), 8*(c%2)+8). Each core computes its 8 heads' attention and the
partial output projection (its 512 rows of Wo); the host sums the two partials
per batch and adds the bias.

On-core dataflow (bf16 matmul operands, fp32 PSUM accumulation), emitted as a
single software pipeline so the QKV projections, PV matmuls and the output
projection all run in the ACT-engine shadow of the exp stream:

  - Q/K stored packed per head-pair: qt/kt[128, hp, S] with head 2hp in
    partitions 0:64 and head 2hp+1 in 64:128 (no zero padding).  Score
    matmuls contract K=64 and run TWO AT A TIME in different PE row groups
    (tile_position (0,0)/(64,0)) writing the two column halves (= two PSUM
    banks) of one [128, 1024] tile, so both heads' scoresT for one t-tile
    cost ~512 PE cycles together.
  - One ACT exp per t-tile covers both heads ([128, 1024], scale fused);
    causal masking multiplies the exp output by a per-rel mask on DVE for
    diagonal tiles only.
  - V stored [128, st, head, 65] = [V | ones]; PV matmuls (m=65) accumulate
    numerators + softmax denominator in a [65, 512] PSUM tile per head.
  - Finalize: one DVE copy [65,512] -> bf16, gpsimd partition_broadcast of
    the denominator row, one DVE divide into outT (no DRAM bounce).
  - Output projection interleaved chunk-major; out is written bf16 and the
    host sums the two half-head partials in fp32 and adds the bias.
"""

import sys

if "/opt/trn_rl_repo" not in sys.path:
    sys.path.insert(0, "/opt/trn_rl_repo")

import math
import numpy as np
from collections import deque
from contextlib import ExitStack

B, S, E, H = 4, 2048, 1024, 16
DH = E // H          # 64
NCORES = 8
NH = 8               # local heads per core
HP = NH // 2         # head pairs
P = 128
NE = E // P          # 8 e-tiles
NT = S // P          # 16 t-tiles
CH = 512
NCH = S // CH        # 4 q-chunks
VW = 128             # V tile cols: [ones | 63 zeros | V(64)]
VO = 64              # V column offset (naturally aligned partition reads)
SCALE = 1.0 / 8.0    # 1/sqrt(DH)
PT_BUFS = 20

_CACHE = {}


def _build_nc():
    import concourse.mybir as mybir
    import concourse.tile as tile
    import concourse.bass as bass
    from concourse import bacc

    f32 = mybir.dt.float32
    bf16 = mybir.dt.bfloat16
    Exp = mybir.ActivationFunctionType.Exp
    Div = mybir.AluOpType.divide
    PSUM = bass.MemorySpace.PSUM

    nc = bacc.Bacc(None)
    x_d = nc.dram_tensor("x", [E, S], bf16, kind="ExternalInput")
    wqk_d = nc.dram_tensor("wqk", [2, HP, P, NE * P], bf16, kind="ExternalInput")
    wv_d = nc.dram_tensor("wv", [P, NE, NH * DH], bf16, kind="ExternalInput")
    wo_d = nc.dram_tensor("wo", [2, P, HP * CH], bf16, kind="ExternalInput")
    mask_d = nc.dram_tensor("mask", [P, NCH, 2 * CH], bf16, kind="ExternalInput")
    out_d = nc.dram_tensor("out", [S, E], bf16, kind="ExternalOutput")

    with ExitStack() as ctx:
        tc = ctx.enter_context(tile.TileContext(nc))
        persist = ctx.enter_context(tc.tile_pool(name="persist", bufs=1))

        qt = persist.tile([P, HP, S], bf16, tag="qt")
        kt = persist.tile([P, HP, S], bf16, tag="kt")
        vf = persist.tile([P, NT, NH, VW], bf16, tag="vf")
        msk = persist.tile([P, NCH, 2 * CH], bf16, tag="msk")
        outTs = [persist.tile([P, S], bf16, tag=f"outT{i}", name="outT")
                 for i in range(HP)]
        xts = []
        wts = {}
        wvt = persist.tile([P, NE, NH * DH], bf16, tag="wvt")
        wt2s = []

        # ---- input DMAs ----
        # SP queue: wv, x tiles 0..3;  ACT queue: wq/wk, x tiles 4..7, wo, mask
        nc.gpsimd.memset(vf[:, :, :, 0:1], 1.0)   # PV row 0 = softmax denom
        nc.gpsimd.memset(vf[:, :, :, 1:VO], 0.0)  # zero pad columns
        # x lands chunk-column-major so chunk-0 projections start after ~1MB:
        # sync queue gets x tiles 0..3, scalar queue x tiles 4..7 plus the
        # Q/K weights for each head pair right after the chunk they unblock;
        # the mask rides the otherwise-idle DVE queue.
        xts = [persist.tile([P, S], bf16, tag=f"xt{et}", name="xt")
               for et in range(NE)]
        for wi in range(2):
            for hp in range(HP):
                wts[(wi, hp)] = persist.tile(
                    [P, NE, P], bf16, tag=f"wt{wi}{hp}", name="wt")
        nc.gpsimd.dma_start(out=msk, in_=mask_d[:])
        for c in range(NCH):
            cs = slice(c * CH, (c + 1) * CH)
            for et in range(NE):
                eng = nc.sync if et < 4 else nc.scalar
                eng.dma_start(out=xts[et][:, cs], in_=x_d[et * P:(et + 1) * P, cs])
            if c < HP:
                for wi in range(2):
                    nc.scalar.dma_start(
                        out=wts[(wi, c)],
                        in_=wqk_d[wi, c].rearrange("p (a b) -> p a b", a=NE))
            if c == 0:
                nc.sync.dma_start(out=wvt, in_=wv_d[:, :, :])
        for ech in range(2):
            wt2 = persist.tile([P, HP, CH], bf16, tag=f"wt2{ech}", name="wt2")
            nc.scalar.dma_start(
                out=wt2, in_=wo_d[ech].rearrange("p (a b) -> p a b", a=HP))
            wt2s.append(wt2)

        wup = persist.tile([P, 2 * CH], bf16, tag="wup")
        nc.gpsimd.memset(wup, 0.0)

        ptp = ctx.enter_context(tc.tile_pool(name="ptp", bufs=PT_BUFS))
        pop = ctx.enter_context(tc.tile_pool(name="pop", bufs=2))
        bcp = ctx.enter_context(tc.tile_pool(name="bcp", bufs=2))
        osb = ctx.enter_context(tc.tile_pool(name="osb", bufs=4))
        scp = ctx.enter_context(tc.tile_pool(name="scp", bufs=2, space=PSUM))
        bk1 = ctx.enter_context(tc.tile_pool(name="bk1", bufs=1, space=PSUM))

        # ---------- PE filler fifo: (key, [step callables]) ----------
        fifo = deque()
        fifo_steps = 0

        def enq(key, steps):
            nonlocal fifo_steps
            fifo.append((key, list(steps)))
            fifo_steps += len(steps)

        def pop_step():
            nonlocal fifo_steps
            while fifo:
                key, steps = fifo[0]
                if not steps:
                    fifo.popleft()
                    continue
                steps.pop(0)()
                fifo_steps -= 1
                return True
            return False

        def flush_until(key):
            # drain chains until (and including) the LAST chain tagged `key`
            nonlocal fifo_steps
            if not any(k == key for k, _ in fifo):
                return
            last = max(i for i, (k, _) in enumerate(fifo) if k == key)
            for _ in range(last + 1):
                k, steps = fifo.popleft()
                for s in steps:
                    s()
                fifo_steps -= len(steps)

        # ---------- projection chains ----------
        prj_i = [0]

        def qk_chain(wi, hp, c):
            # Q (wi=0) or K (wi=1) projection for head pair hp, chunk c
            dst = qt if wi == 0 else kt
            wt = wts[(wi, hp)]
            steps = []
            pst = {}

            def mk_mm(et):
                def f():
                    if et == 0:
                        pst["t"] = bk1.tile([P, CH], f32, tag="proj",
                                            bufs=2, name="proj")
                    nc.tensor.matmul(
                        pst["t"], wt[:, et, :],
                        xts[et][:, c * CH:(c + 1) * CH],
                        start=(et == 0), stop=(et == NE - 1),
                        skip_group_check=True)
                return f

            for et in range(NE):
                steps.append(mk_mm(et))

            def cp():
                nc.vector.tensor_copy(
                    out=dst[:, hp, c * CH:(c + 1) * CH], in_=pst["t"])
            steps.append(cp)
            return steps

        def v_chain(st):
            steps = []
            pst = {}

            def mk_mm(et):
                def f():
                    if et == 0:
                        pst["t"] = bk1.tile([P, CH], f32, tag="proj",
                                            bufs=2, name="proj")
                    nc.tensor.matmul(
                        pst["t"], xts[et][:, st * P:(st + 1) * P], wvt[:, et, :],
                        start=(et == 0), stop=(et == NE - 1),
                        skip_group_check=True)
                return f

            for et in range(NE):
                steps.append(mk_mm(et))

            def cp():
                nc.vector.tensor_copy(
                    out=vf[:, st, :, VO:VO + DH],
                    in_=pst["t"].rearrange("p (h d) -> p h d", h=NH))
            steps.append(cp)
            return steps

        def p4_chain(c, ech, st):
            steps = []
            pst = {}

            def mk_mm(hp):
                def f():
                    if hp == 0:
                        pst["t"] = bk1.tile([P, CH], f32, tag="proj",
                                            bufs=2, name="proj")
                    nc.tensor.matmul(
                        pst["t"], outTs[hp][:, st * P:(st + 1) * P],
                        wt2s[ech][:, hp, :],
                        start=(hp == 0), stop=(hp == HP - 1),
                        skip_group_check=True)
                return f

            for hp in range(HP):
                steps.append(mk_mm(hp))

            def cp():
                ob = osb.tile([P, CH], bf16, tag="ob", name="ob")
                if c == NCH - 1:
                    nc.scalar.copy(out=ob, in_=pst["t"])
                else:
                    nc.vector.tensor_copy(out=ob, in_=pst["t"])
                oeng = nc.scalar if (c == NCH - 1 and st % 2) else nc.sync
                oeng.dma_start(
                    out=out_d[st * P:(st + 1) * P, ech * CH:(ech + 1) * CH],
                    in_=ob)
            steps.append(cp)
            return steps

        # enqueue all projection chains in priority order
        for hp in range(HP):
            for wi in range(2):
                enq(("qk", hp, 0), qk_chain(wi, hp, 0))
        for st in range(4):
            enq(("v", st), v_chain(st))
        for c in range(1, NCH):
            for hp in range(HP):
                for wi in range(2):
                    enq(("qk", hp, c), qk_chain(wi, hp, c))
            for st in range(4 * c, 4 * c + 4):
                enq(("v", st), v_chain(st))

        # PE warm-up during the input-DMA wait (results discarded)
        wps = bk1.tile([P, CH], f32, tag="proj", bufs=2, name="wps")
        for _ in range(24):
            nc.tensor.matmul(wps, wup[:, 0:P], wup[:, P:P + CH],
                             start=True, stop=True, skip_group_check=True)

        # ---------- attention pipeline ----------
        units = [(c, hp) for c in range(NCH) for hp in range(HP)]
        total_steps = sum(4 * c + 4 for c, _ in units)
        steps_done = [0]

        p4_pending = []

        def emit_finalize(c, hp, pvs):
            ccols = slice(c * CH, (c + 1) * CH)
            if hp == HP - 1:
                for ech in range(2):
                    for st in range(4 * c, 4 * c + 4):
                        p4_pending.append((("p4", c, ech, st),
                                           p4_chain(c, ech, st)))
            for h in range(2):
                po = pop.tile([VW, CH], f32, tag="po", name="po")
                nc.vector.tensor_copy(out=po, in_=pvs[h][0:VW, :])
                nc.vector.reciprocal_approx_fast(out=po[0:1, :], in_=po[0:1, :])
                bc = bcp.tile([P, CH], f32, tag="bc", name="bc")
                nc.gpsimd.partition_broadcast(bc, po[0:1, :], channels=P)
                nc.vector.tensor_mul(
                    outTs[hp][h * DH:(h + 1) * DH, ccols],
                    po[VO:VO + DH, :], bc[VO:VO + DH, :])

        pend = None      # (c, hp, pts)
        fin_q = deque()  # (c, hp, pvs) awaiting finalize
        for c, hp in units:
            while p4_pending:
                enq(*p4_pending.pop(0))
            flush_until(("qk", hp, c))
            if fin_q:
                emit_finalize(*fin_q.popleft())
            ntv = 4 * c + 4
            # build the pending unit's PV matmul list
            pv_mms = []
            pvs = None
            if pend is not None:
                pc, php, ppts = pend
                flush_until(("v", 4 * pc + 3))
                pvs = {h: bk1.tile([P, CH], f32, tag=f"pv{h}", bufs=1,
                                        name="pv")
                       for h in range(2)}
                pntv = 4 * pc + 4

                def mk_pv(h, tt, pc=pc, php=php, ppts=ppts, pvs=pvs):
                    pntv_ = 4 * pc + 4

                    def f():
                        nc.tensor.matmul(
                            pvs[h][0:VW, :],
                            vf[:, tt, 2 * php + h, :],
                            ppts[tt][:, h * CH:(h + 1) * CH],
                            start=(tt == 0), stop=(tt == pntv_ - 1),
                            skip_group_check=True)
                    return f

                for tt in range(pntv):
                    for h in range(2):
                        pv_mms.append(mk_pv(h, tt))
            done = 0

            pts = []
            for tt in range(ntv):
                sps = scp.tile([P, 2 * CH], f32, tag="sp", name="sp")
                for h in range(2):
                    hl = h * DH
                    nc.tensor.matmul(
                        sps[:, h * CH:(h + 1) * CH],
                        kt[hl:hl + DH, hp, tt * P:(tt + 1) * P],
                        qt[hl:hl + DH, hp, c * CH:(c + 1) * CH],
                        start=True, stop=True, skip_group_check=True)
                pt = ptp.tile([P, 2 * CH], bf16, tag="pt", name="pt")
                nc.scalar.activation(out=pt, in_=sps, func=Exp, scale=SCALE)
                rel = tt - 4 * c
                if rel >= 0:
                    nc.vector.tensor_mul(pt, pt, msk[:, rel, :])
                pts.append(pt)
                steps_done[0] += 1
                # interleave PV of the pending unit
                want = (tt + 1) * len(pv_mms) // ntv
                while done < want:
                    pv_mms[done]()
                    done += 1
                # paced fillers (projections / output projection)
                left = total_steps - steps_done[0]
                if left > 0:
                    k = -(-fifo_steps // left)
                    for _ in range(min(k, 12)):
                        if not pop_step():
                            break
            while done < len(pv_mms):
                pv_mms[done]()
                done += 1
            if pend is not None:
                fin_q.append((pend[0], pend[1], pvs))
                # after the last unit of chunk row c' completes PV+finalize,
                # its P4 becomes available; enqueue when finalize emitted
            pend = (c, hp, pts)
            # enqueue P4 chains once the last head-pair of a chunk is finalized
            # (handled below after finalize emission)
            if fin_q and fin_q[0][1] == HP - 1:
                pass  # P4 enqueue happens right after its finalize pops

            # check if a finalize for hp==HP-1 was just emitted this unit
            # (P4 enqueue logic lives where finalize is popped)

        # drain: PV for the last unit
        if pend is not None:
            pc, php, ppts = pend
            flush_until(("v", NT - 1))
            pvs = {h: bk1.tile([P, CH], f32, tag=f"pv{h}", bufs=1,
                                    name="pv")
                   for h in range(2)}
            pntv = 4 * pc + 4
            for tt in range(pntv):
                for h in range(2):
                    nc.tensor.matmul(
                        pvs[h][0:VW, :],
                        vf[:, tt, 2 * php + h, :],
                        ppts[tt][:, h * CH:(h + 1) * CH],
                        start=(tt == 0), stop=(tt == pntv - 1),
                        skip_group_check=True)
            fin_q.append((pc, php, pvs))
        while fin_q:
            emit_finalize(*fin_q.popleft())
        while p4_pending:
            enq(*p4_pending.pop(0))
        # remaining fillers (last chunk's P4 and any stragglers)
        while fifo:
            pop_step()

    nc.finalize()
    return nc


def _get_nc():
    if "nc" not in _CACHE:
        _CACHE["nc"] = _build_nc()
    return _CACHE["nc"]


def _make_in_maps(x, Wq, Wk, Wv, Wo):
    import ml_dtypes

    bf = ml_dtypes.bfloat16
    pcol = np.arange(P)[:, None]
    qq = np.arange(CH)[None, :]
    mask_half = np.stack([(pcol <= qq - P * rel) for rel in range(NCH)], axis=1)
    mask = np.concatenate([mask_half, mask_half], axis=2).astype(bf)

    in_maps = []
    for core in range(NCORES):
        b, half = divmod(core, 2)
        hs = slice(half * NH, (half + 1) * NH)
        wqk = np.empty((2, HP, P, NE * P), dtype=bf)
        for wi, W in ((0, Wq), (1, Wk)):
            Wpk = W[hs].transpose(1, 0, 2).reshape(E, NH * DH)
            for hp in range(HP):
                blk = Wpk[:, hp * P:(hp + 1) * P]
                wqk[wi, hp] = (blk.reshape(NE, P, P).transpose(1, 0, 2)
                               .reshape(P, NE * P).astype(bf))
        Wvpk = Wv[hs].transpose(1, 0, 2).reshape(E, NH * DH)
        wv = (Wvpk.reshape(NE, P, NH * DH).transpose(1, 0, 2)
              .reshape(P, NE * NH * DH).astype(bf)).reshape(P, NE, NH * DH)
        Wol = Wo[half * NH * DH:(half + 1) * NH * DH]  # [512, E]
        wo = np.empty((2, P, HP * CH), dtype=bf)
        for ech in range(2):
            blk = Wol[:, ech * CH:(ech + 1) * CH]  # [512, 512]
            wo[ech] = (blk.reshape(HP, P, CH).transpose(1, 0, 2)
                       .reshape(P, HP * CH).astype(bf))
        in_maps.append({
            "x": np.ascontiguousarray(x[b].T.astype(bf)),
            "wqk": wqk,
            "wv": np.ascontiguousarray(wv),
            "wo": wo,
            "mask": np.ascontiguousarray(mask),
        })
    return in_maps


def _ensure_ntff_hook():
    """Register the axon NTFF profile hook under antenv.axon_hooks."""
    import types
    try:
        import antenv.axon_hooks  # noqa: F401
        return
    except ImportError:
        pass
    try:
        from trn_agent_boot.trn_boot import _ntff_profile_via_ctypes
        hook = _ntff_profile_via_ctypes("/opt/axon/libaxon_pjrt.so")
    except Exception:
        hook = None
    mod = types.ModuleType("antenv.axon_hooks")
    mod.get_axon_ntff_profile_hook = lambda: hook
    mod.set_axon_ntff_profile_hook = lambda h: None
    sys.modules["antenv.axon_hooks"] = mod


def _run(inputs, trace=False):
    from concourse.bass_utils import run_bass_kernel_spmd

    if trace:
        _ensure_ntff_hook()

    x = np.asarray(inputs["x"], dtype=np.float32)
    Wq = np.asarray(inputs["Wq"], dtype=np.float32)
    Wk = np.asarray(inputs["Wk"], dtype=np.float32)
    Wv = np.asarray(inputs["Wv"], dtype=np.float32)
    Wo = np.asarray(inputs["Wo"], dtype=np.float32)
    bo = np.asarray(inputs["bo"], dtype=np.float32)

    nc = _get_nc()
    in_maps = _make_in_maps(x, Wq, Wk, Wv, Wo)
    res = run_bass_kernel_spmd(nc, in_maps, list(range(NCORES)), trace=trace)
    out = np.empty((B, S, E), dtype=np.float32)
    for b in range(B):
        out[b] = (res.results[2 * b]["out"].astype(np.float32)
                  + res.results[2 * b + 1]["out"].astype(np.float32) + bo)
    return out, res


def kernel(**inputs):
    out, _ = _run(inputs, trace=False)
    return out


# revision 23
# speedup vs baseline: 1.0275x; 1.0275x over previous
"""Trainium2 Bass kernel for causal MultiHeadAttention (B=4,S=2048,E=1024,H=16).

Sharding: 8 cores = (batch b, head-half) grid. Core c handles batch c//2 and
heads [8*(c%2), 8*(c%2)+8). Each core computes its 8 heads' attention and the
partial output projection (its 512 rows of Wo); the host sums the two partials
per batch and adds the bias.

On-core dataflow (bf16 matmul operands, fp32 PSUM accumulation), emitted as a
single software pipeline so the QKV projections, PV matmuls and the output
projection all run in the ACT-engine shadow of the exp stream:

  - Q/K stored packed per head-pair: qt/kt[128, hp, S] with head 2hp in
    partitions 0:64 and head 2hp+1 in 64:128 (no zero padding).  Score
    matmuls contract K=64 and run TWO AT A TIME in different PE row groups
    (tile_position (0,0)/(64,0)) writing the two column halves (= two PSUM
    banks) of one [128, 1024] tile, so both heads' scoresT for one t-tile
    cost ~512 PE cycles together.
  - One ACT exp per t-tile covers both heads ([128, 1024], scale fused);
    causal masking multiplies the exp output by a per-rel mask on DVE for
    diagonal tiles only.
  - V stored [128, st, head, 65] = [V | ones]; PV matmuls (m=65) accumulate
    numerators + softmax denominator in a [65, 512] PSUM tile per head.
  - Finalize: one DVE copy [65,512] -> bf16, gpsimd partition_broadcast of
    the denominator row, one DVE divide into outT (no DRAM bounce).
  - Output projection interleaved chunk-major; out is written bf16 and the
    host sums the two half-head partials in fp32 and adds the bias.
"""

import sys

if "/opt/trn_rl_repo" not in sys.path:
    sys.path.insert(0, "/opt/trn_rl_repo")

import math
import numpy as np
from collections import deque
from contextlib import ExitStack

B, S, E, H = 4, 2048, 1024, 16
DH = E // H          # 64
NCORES = 8
NH = 8               # local heads per core
HP = NH // 2         # head pairs
P = 128
NE = E // P          # 8 e-tiles
NT = S // P          # 16 t-tiles
CH = 512
NCH = S // CH        # 4 q-chunks
VW = 128             # V tile cols: [ones | 63 zeros | V(64)]
VO = 64              # V column offset (naturally aligned partition reads)
SCALE = 1.0 / 8.0    # 1/sqrt(DH)
PT_BUFS = 20

_CACHE = {}


def _build_nc():
    import concourse.mybir as mybir
    import concourse.tile as tile
    import concourse.bass as bass
    from concourse import bacc

    f32 = mybir.dt.float32
    bf16 = mybir.dt.bfloat16
    Exp = mybir.ActivationFunctionType.Exp
    Div = mybir.AluOpType.divide
    PSUM = bass.MemorySpace.PSUM

    nc = bacc.Bacc(None)
    x_d = nc.dram_tensor("x", [E, S], bf16, kind="ExternalInput")
    wqk_d = nc.dram_tensor("wqk", [2, HP, P, NE * P], bf16, kind="ExternalInput")
    wv_d = nc.dram_tensor("wv", [P, NE, NH * DH], bf16, kind="ExternalInput")
    wo_d = nc.dram_tensor("wo", [2, P, HP * CH], bf16, kind="ExternalInput")
    mask_d = nc.dram_tensor("mask", [P, NCH, 2 * CH], bf16, kind="ExternalInput")
    out_d = nc.dram_tensor("out", [S, E], bf16, kind="ExternalOutput")

    with ExitStack() as ctx:
        tc = ctx.enter_context(tile.TileContext(nc))
        persist = ctx.enter_context(tc.tile_pool(name="persist", bufs=1))

        qt = persist.tile([P, HP, S], bf16, tag="qt")
        kt = persist.tile([P, HP, S], bf16, tag="kt")
        vf = persist.tile([P, NT, NH, VW], bf16, tag="vf")
        msk = persist.tile([P, NCH, 2 * CH], bf16, tag="msk")
        outTs = [persist.tile([P, S], bf16, tag=f"outT{i}", name="outT")
                 for i in range(HP)]
        xts = []
        wts = {}
        wvt = persist.tile([P, NE, NH * DH], bf16, tag="wvt")
        wt2s = []

        # ---- input DMAs ----
        # SP queue: wv, x tiles 0..3;  ACT queue: wq/wk, x tiles 4..7, wo, mask
        # x lands chunk-column-major so chunk-0 projections start after ~1MB:
        # sync queue gets x tiles 0..3, scalar queue x tiles 4..7 plus the
        # Q/K weights for each head pair right after the chunk they unblock.
        # The latest-needed loads (mask, x chunk 3 low tiles) ride the gpsimd
        # software-DGE as a third DMA channel.
        xts = [persist.tile([P, S], bf16, tag=f"xt{et}", name="xt")
               for et in range(NE)]
        for wi in range(2):
            for hp in range(HP):
                wts[(wi, hp)] = persist.tile(
                    [P, NE, P], bf16, tag=f"wt{wi}{hp}", name="wt")
        nc.gpsimd.dma_start(out=msk, in_=mask_d[:])
        c3 = slice(3 * CH, 4 * CH)
        for et in range(4):
            nc.gpsimd.dma_start(out=xts[et][:, c3],
                                in_=x_d[et * P:(et + 1) * P, c3])
        nc.gpsimd.memset(vf[:, :, :, 0:1], 1.0)   # PV row 0 = softmax denom
        nc.gpsimd.memset(vf[:, :, :, 1:VO], 0.0)  # zero pad columns
        for c in range(NCH):
            cs = slice(c * CH, (c + 1) * CH)
            for et in range(NE):
                if c == 3 and et < 4:
                    continue  # on the gpsimd queue
                eng = nc.sync if et < 4 else nc.scalar
                eng.dma_start(out=xts[et][:, cs], in_=x_d[et * P:(et + 1) * P, cs])
            if c < HP:
                for wi in range(2):
                    nc.scalar.dma_start(
                        out=wts[(wi, c)],
                        in_=wqk_d[wi, c].rearrange("p (a b) -> p a b", a=NE))
            if c == 0:
                nc.sync.dma_start(out=wvt, in_=wv_d[:, :, :])
        for ech in range(2):
            wt2 = persist.tile([P, HP, CH], bf16, tag=f"wt2{ech}", name="wt2")
            nc.scalar.dma_start(
                out=wt2, in_=wo_d[ech].rearrange("p (a b) -> p a b", a=HP))
            wt2s.append(wt2)

        ptp = ctx.enter_context(tc.tile_pool(name="ptp", bufs=PT_BUFS))
        pop = ctx.enter_context(tc.tile_pool(name="pop", bufs=2))
        bcp = ctx.enter_context(tc.tile_pool(name="bcp", bufs=2))
        osb = ctx.enter_context(tc.tile_pool(name="osb", bufs=4))
        scp = ctx.enter_context(tc.tile_pool(name="scp", bufs=2, space=PSUM))
        bk1 = ctx.enter_context(tc.tile_pool(name="bk1", bufs=1, space=PSUM))

        # ---------- PE filler fifo: (key, [step callables]) ----------
        fifo = deque()
        fifo_steps = 0

        def enq(key, steps):
            nonlocal fifo_steps
            fifo.append((key, list(steps)))
            fifo_steps += len(steps)

        def pop_step():
            nonlocal fifo_steps
            while fifo:
                key, steps = fifo[0]
                if not steps:
                    fifo.popleft()
                    continue
                steps.pop(0)()
                fifo_steps -= 1
                return True
            return False

        def flush_until(key):
            # drain chains until (and including) the LAST chain tagged `key`
            nonlocal fifo_steps
            if not any(k == key for k, _ in fifo):
                return
            last = max(i for i, (k, _) in enumerate(fifo) if k == key)
            for _ in range(last + 1):
                k, steps = fifo.popleft()
                for s in steps:
                    s()
                fifo_steps -= len(steps)

        # ---------- projection chains ----------
        prj_i = [0]

        def qk_chain(wi, hp, c):
            # Q (wi=0) or K (wi=1) projection for head pair hp, chunk c
            dst = qt if wi == 0 else kt
            wt = wts[(wi, hp)]
            steps = []
            pst = {}

            def mk_mm(et):
                def f():
                    if et == 0:
                        pst["t"] = bk1.tile([P, CH], f32, tag="proj",
                                            bufs=2, name="proj")
                    nc.tensor.matmul(
                        pst["t"], wt[:, et, :],
                        xts[et][:, c * CH:(c + 1) * CH],
                        start=(et == 0), stop=(et == NE - 1),
                        skip_group_check=True)
                return f

            for et in range(NE):
                steps.append(mk_mm(et))

            def cp():
                nc.vector.tensor_copy(
                    out=dst[:, hp, c * CH:(c + 1) * CH], in_=pst["t"])
            steps.append(cp)
            return steps

        def v_chain(st):
            steps = []
            pst = {}

            def mk_mm(et):
                def f():
                    if et == 0:
                        pst["t"] = bk1.tile([P, CH], f32, tag="proj",
                                            bufs=2, name="proj")
                    nc.tensor.matmul(
                        pst["t"], xts[et][:, st * P:(st + 1) * P], wvt[:, et, :],
                        start=(et == 0), stop=(et == NE - 1),
                        skip_group_check=True)
                return f

            for et in range(NE):
                steps.append(mk_mm(et))

            def cp():
                nc.vector.tensor_copy(
                    out=vf[:, st, :, VO:VO + DH],
                    in_=pst["t"].rearrange("p (h d) -> p h d", h=NH))
            steps.append(cp)
            return steps

        def p4_chain(c, ech, st):
            steps = []
            pst = {}

            def mk_mm(hp):
                def f():
                    if hp == 0:
                        pst["t"] = bk1.tile([P, CH], f32, tag="proj",
                                            bufs=2, name="proj")
                    nc.tensor.matmul(
                        pst["t"], outTs[hp][:, st * P:(st + 1) * P],
                        wt2s[ech][:, hp, :],
                        start=(hp == 0), stop=(hp == HP - 1),
                        skip_group_check=True)
                return f

            for hp in range(HP):
                steps.append(mk_mm(hp))

            def cp():
                ob = osb.tile([P, CH], bf16, tag="ob", name="ob")
                if c == NCH - 1:
                    nc.scalar.copy(out=ob, in_=pst["t"])
                else:
                    nc.vector.tensor_copy(out=ob, in_=pst["t"])
                oeng = nc.scalar if (c == NCH - 1 and st % 2) else nc.sync
                oeng.dma_start(
                    out=out_d[st * P:(st + 1) * P, ech * CH:(ech + 1) * CH],
                    in_=ob)
            steps.append(cp)
            return steps

        # enqueue all projection chains in priority order
        for hp in range(HP):
            for wi in range(2):
                enq(("qk", hp, 0), qk_chain(wi, hp, 0))
        for st in range(4):
            enq(("v", st), v_chain(st))
        for c in range(1, NCH):
            for hp in range(HP):
                for wi in range(2):
                    enq(("qk", hp, c), qk_chain(wi, hp, c))
            for st in range(4 * c, 4 * c + 4):
                enq(("v", st), v_chain(st))

        # ---------- attention pipeline ----------
        units = [(c, hp) for c in range(NCH) for hp in range(HP)]
        total_steps = sum(4 * c + 4 for c, _ in units)
        steps_done = [0]

        p4_pending = []

        def emit_finalize(c, hp, pvs):
            ccols = slice(c * CH, (c + 1) * CH)
            if hp == HP - 1:
                for ech in range(2):
                    for st in range(4 * c, 4 * c + 4):
                        p4_pending.append((("p4", c, ech, st),
                                           p4_chain(c, ech, st)))
            for h in range(2):
                po = pop.tile([VW, CH], f32, tag="po", name="po")
                nc.vector.tensor_copy(out=po, in_=pvs[h][0:VW, :])
                nc.vector.reciprocal_approx_fast(out=po[0:1, :], in_=po[0:1, :])
                bc = bcp.tile([P, CH], f32, tag="bc", name="bc")
                nc.gpsimd.partition_broadcast(bc, po[0:1, :], channels=P)
                nc.vector.tensor_mul(
                    outTs[hp][h * DH:(h + 1) * DH, ccols],
                    po[VO:VO + DH, :], bc[VO:VO + DH, :])

        pend = None      # (c, hp, pts)
        fin_q = deque()  # (c, hp, pvs) awaiting finalize
        for c, hp in units:
            while p4_pending:
                enq(*p4_pending.pop(0))
            flush_until(("qk", hp, c))
            if fin_q:
                emit_finalize(*fin_q.popleft())
            ntv = 4 * c + 4
            # build the pending unit's PV matmul list
            pv_mms = []
            pvs = None
            if pend is not None:
                pc, php, ppts = pend
                flush_until(("v", 4 * pc + 3))
                pvs = {h: bk1.tile([P, CH], f32, tag=f"pv{h}", bufs=1,
                                        name="pv")
                       for h in range(2)}
                pntv = 4 * pc + 4

                def mk_pv(h, tt, pc=pc, php=php, ppts=ppts, pvs=pvs):
                    pntv_ = 4 * pc + 4

                    def f():
                        nc.tensor.matmul(
                            pvs[h][0:VW, :],
                            vf[:, tt, 2 * php + h, :],
                            ppts[tt][:, h * CH:(h + 1) * CH],
                            start=(tt == 0), stop=(tt == pntv_ - 1),
                            skip_group_check=True)
                    return f

                for tt in range(pntv):
                    for h in range(2):
                        pv_mms.append(mk_pv(h, tt))
            done = 0

            pts = []
            for tt in range(ntv):
                sps = scp.tile([P, 2 * CH], f32, tag="sp", name="sp")
                for h in range(2):
                    hl = h * DH
                    nc.tensor.matmul(
                        sps[:, h * CH:(h + 1) * CH],
                        kt[hl:hl + DH, hp, tt * P:(tt + 1) * P],
                        qt[hl:hl + DH, hp, c * CH:(c + 1) * CH],
                        start=True, stop=True, skip_group_check=True)
                pt = ptp.tile([P, 2 * CH], bf16, tag="pt", name="pt")
                nc.scalar.activation(out=pt, in_=sps, func=Exp, scale=SCALE)
                rel = tt - 4 * c
                if rel >= 0:
                    nc.vector.tensor_mul(pt, pt, msk[:, rel, :])
                pts.append(pt)
                steps_done[0] += 1
                # interleave PV of the pending unit
                want = (tt + 1) * len(pv_mms) // ntv
                while done < want:
                    pv_mms[done]()
                    done += 1
                # paced fillers (projections / output projection)
                left = total_steps - steps_done[0]
                if left > 0:
                    k = -(-fifo_steps // left)
                    for _ in range(min(k, 8)):
                        if not pop_step():
                            break
            while done < len(pv_mms):
                pv_mms[done]()
                done += 1
            if pend is not None:
                fin_q.append((pend[0], pend[1], pvs))
                # after the last unit of chunk row c' completes PV+finalize,
                # its P4 becomes available; enqueue when finalize emitted
            pend = (c, hp, pts)
            # enqueue P4 chains once the last head-pair of a chunk is finalized
            # (handled below after finalize emission)
            if fin_q and fin_q[0][1] == HP - 1:
                pass  # P4 enqueue happens right after its finalize pops

            # check if a finalize for hp==HP-1 was just emitted this unit
            # (P4 enqueue logic lives where finalize is popped)

        # drain: PV for the last unit
        if pend is not None:
            pc, php, ppts = pend
            flush_until(("v", NT - 1))
            pvs = {h: bk1.tile([P, CH], f32, tag=f"pv{h}", bufs=1,
                                    name="pv")
                   for h in range(2)}
            pntv = 4 * pc + 4
            for tt in range(pntv):
                for h in range(2):
                    nc.tensor.matmul(
                        pvs[h][0:VW, :],
                        vf[:, tt, 2 * php + h, :],
                        ppts[tt][:, h * CH:(h + 1) * CH],
                        start=(tt == 0), stop=(tt == pntv - 1),
                        skip_group_check=True)
            fin_q.append((pc, php, pvs))
        while fin_q:
            emit_finalize(*fin_q.popleft())
        while p4_pending:
            enq(*p4_pending.pop(0))
        # remaining fillers (last chunk's P4 and any stragglers)
        while fifo:
            pop_step()

    nc.finalize()
    return nc


def _get_nc():
    if "nc" not in _CACHE:
        _CACHE["nc"] = _build_nc()
    return _CACHE["nc"]


def _make_in_maps(x, Wq, Wk, Wv, Wo):
    import ml_dtypes

    bf = ml_dtypes.bfloat16
    pcol = np.arange(P)[:, None]
    qq = np.arange(CH)[None, :]
    mask_half = np.stack([(pcol <= qq - P * rel) for rel in range(NCH)], axis=1)
    mask = np.concatenate([mask_half, mask_half], axis=2).astype(bf)

    in_maps = []
    for core in range(NCORES):
        b, half = divmod(core, 2)
        hs = slice(half * NH, (half + 1) * NH)
        wqk = np.empty((2, HP, P, NE * P), dtype=bf)
        for wi, W in ((0, Wq), (1, Wk)):
            Wpk = W[hs].transpose(1, 0, 2).reshape(E, NH * DH)
            for hp in range(HP):
                blk = Wpk[:, hp * P:(hp + 1) * P]
                wqk[wi, hp] = (blk.reshape(NE, P, P).transpose(1, 0, 2)
                               .reshape(P, NE * P).astype(bf))
        Wvpk = Wv[hs].transpose(1, 0, 2).reshape(E, NH * DH)
        wv = (Wvpk.reshape(NE, P, NH * DH).transpose(1, 0, 2)
              .reshape(P, NE * NH * DH).astype(bf)).reshape(P, NE, NH * DH)
        Wol = Wo[half * NH * DH:(half + 1) * NH * DH]  # [512, E]
        wo = np.empty((2, P, HP * CH), dtype=bf)
        for ech in range(2):
            blk = Wol[:, ech * CH:(ech + 1) * CH]  # [512, 512]
            wo[ech] = (blk.reshape(HP, P, CH).transpose(1, 0, 2)
                       .reshape(P, HP * CH).astype(bf))
        in_maps.append({
            "x": np.ascontiguousarray(x[b].T.astype(bf)),
            "wqk": wqk,
            "wv": np.ascontiguousarray(wv),
            "wo": wo,
            "mask": np.ascontiguousarray(mask),
        })
    return in_maps


def _ensure_ntff_hook():
    """Register the axon NTFF profile hook under antenv.axon_hooks."""
    import types
    try:
        import antenv.axon_hooks  # noqa: F401
        return
    except ImportError:
        pass
    try:
        from trn_agent_boot.trn_boot import _ntff_profile_via_ctypes
        hook = _ntff_profile_via_ctypes("/opt/axon/libaxon_pjrt.so")
    except Exception:
        hook = None
    mod = types.ModuleType("antenv.axon_hooks")
    mod.get_axon_ntff_profile_hook = lambda: hook
    mod.set_axon_ntff_profile_hook = lambda h: None
    sys.modules["antenv.axon_hooks"] = mod


def _run(inputs, trace=False):
    from concourse.bass_utils import run_bass_kernel_spmd

    if trace:
        _ensure_ntff_hook()

    x = np.asarray(inputs["x"], dtype=np.float32)
    Wq = np.asarray(inputs["Wq"], dtype=np.float32)
    Wk = np.asarray(inputs["Wk"], dtype=np.float32)
    Wv = np.asarray(inputs["Wv"], dtype=np.float32)
    Wo = np.asarray(inputs["Wo"], dtype=np.float32)
    bo = np.asarray(inputs["bo"], dtype=np.float32)

    nc = _get_nc()
    in_maps = _make_in_maps(x, Wq, Wk, Wv, Wo)
    res = run_bass_kernel_spmd(nc, in_maps, list(range(NCORES)), trace=trace)
    out = np.empty((B, S, E), dtype=np.float32)
    for b in range(B):
        out[b] = (res.results[2 * b]["out"].astype(np.float32)
                  + res.results[2 * b + 1]["out"].astype(np.float32) + bo)
    return out, res


def kernel(**inputs):
    out, _ = _run(inputs, trace=False)
    return out
